# revision 3
# baseline (speedup 1.0000x reference)
"""MoE FeedForward (top-2 of 4 experts) — expert-parallel Trainium2 kernel.

Strategy (matches the sharding hint): the tiny gating matmul + top-k routing
run on host as part of input sharding; tokens are dispatched by gate index to
expert-owning cores (expert e -> cores 2e, 2e+1). Each core computes
    y^T = (relu(W1^T @ x^T + b1) -> W2^T @ mid + b2) * gate
entirely in transposed [feature, token] layout (no on-device transposes),
with bf16 matmuls accumulating in fp32 PSUM. The host combine scatter-adds
the two gate-weighted expert contributions per token.

Capacity is FIXED at C=2048 columns per core (the exact balanced share of
N_TOKENS*TOP_K/8): experts with more than 2C routed pairs spill their
lowest-gate pairs to a tiny host-side numpy residual (exact, ~tens of pairs),
lighter experts are zero-padded. A single compiled program therefore serves
every input and every core runs identical, perfectly balanced work.

Fast path (used whenever b1 == b2 == 0, which holds for this problem): the
positive gate is folded into x on host (relu(g*z) == g*relu(z) for g > 0),
removing the on-device gate multiply and its [128, C] gate-broadcast load.

Per-core schedule: chunks of [256, 512, 512, 512, 256] columns. The small
first chunk needs only 0.5 MB (x0 + first two w1 h-tiles) before matmuls can
start, and its weight consumption rate stays below DMA delivery during the
bandwidth-saturated head; the small last chunk shrinks the drain tail.
Weights stream on the sync DMA ring, activations on the scalar ring and
outputs (bf16) on the vector ring so issue latencies overlap. w1 is packed
h-tile-major so arrival order equals consumption order.

Model dims (hardcoded per problem spec): N=8192 tokens, D=512, H=2048,
E=4 experts, top-k=2, 8 NeuronCores.
"""

import numpy as np
import ml_dtypes
from contextlib import ExitStack

D = 512
H = 2048
E = 4
TOP_K = 2
N_CORES = 8
ND = D // 128    # 4 d-tiles
NH = H // 128    # 16 h-tiles
C = 2048         # fixed per-core token capacity
CHUNKS = [(0, 256), (256, 512), (768, 512), (1280, 512), (1792, 256)]
N_WARM = 40      # PE warm-up matmuls (cover preamble->first-data window)

_NC_CACHE = {}


def _build_moe_nc(fold_gate: bool):
    """Per-core SPMD program: [D,C] bf16 tokens -> [D,C] bf16 expert output."""
    import concourse.mybir as mybir
    from concourse import bacc, tile

    dt = mybir.dt
    AF = mybir.ActivationFunctionType

    chunks = CHUNKS
    nc = bacc.Bacc(None, target_bir_lowering=False)

    # host pre-arranges every input partition-major so each DMA below is a
    # flat, fully contiguous [128, K] copy (max SDMA bandwidth, min
    # descriptor count):
    #   w1h/w1b/w1r: h-tile-major blocks, per ht: [128, ND*128] (di-major)
    #     -> arrival order == gemm1 consumption order (ht ascending)
    #   xt: chunk-major blocks, inside a block di-major: [128, ND*S]
    #   w2: wb-major blocks of [128, 8*512]
    w1h = nc.dram_tensor("w1h", [128, 2 * ND * 128], dt.bfloat16,
                         kind="ExternalInput")           # ht 0-1
    w1b = nc.dram_tensor("w1b", [128, 2 * ND * 128], dt.bfloat16,
                         kind="ExternalInput")           # ht 2-3
    w1r = nc.dram_tensor("w1r", [128, 12 * ND * 128], dt.bfloat16,
                         kind="ExternalInput")           # ht 4-15
    w2 = nc.dram_tensor("w2", [128, 2 * 8 * 512], dt.bfloat16,
                        kind="ExternalInput")
    xt = nc.dram_tensor("xt", [128, ND * C], dt.bfloat16,
                        kind="ExternalInput")
    if not fold_gate:
        b1r = nc.dram_tensor("b1r", [128, NH], dt.float32, kind="ExternalInput")
        b2r = nc.dram_tensor("b2r", [128, ND], dt.float32, kind="ExternalInput")
        gr = nc.dram_tensor("gr", [128, C], dt.float32, kind="ExternalInput")
    # output, chunk-major like xt: per chunk a [128, ND*S] contiguous block
    # (one DMA per (chunk, di)); host unpacks back to [N, D] token rows
    yt = nc.dram_tensor("yt", [128, ND * C], dt.bfloat16, kind="ExternalOutput")

    # flat offset of each chunk's block inside xt / yt
    xy_off = {}
    acc = 0
    for (c0, S) in chunks:
        xy_off[c0] = acc
        acc += ND * S

    with tile.TileContext(nc) as tc, ExitStack() as ctx:
        wpool = ctx.enter_context(tc.tile_pool(name="weights", bufs=1))
        xpool = ctx.enter_context(tc.tile_pool(name="x", bufs=1))
        midp = ctx.enter_context(tc.tile_pool(name="mid", bufs=34))
        p1 = ctx.enter_context(tc.tile_pool(name="p1", bufs=4, space="PSUM"))
        p2 = ctx.enter_context(tc.tile_pool(name="p2", bufs=3, space="PSUM"))
        pw = ctx.enter_context(tc.tile_pool(name="pw", bufs=1, space="PSUM"))
        ypool = ctx.enter_context(tc.tile_pool(name="y", bufs=6))

        # PE warm-up: dummy matmuls spanning the engine preamble + first
        # input DMAs so the HAM clock gate is at full rate when real matmuls
        # begin. Output bank is never read.
        warm_sb = wpool.tile([128, 128], dt.bfloat16, tag="warm", name="warm_sb")
        nc.vector.memset(warm_sb[:], 0.0)
        warm_ps = pw.tile([128, 64], dt.float32, tag="warm_ps", name="warm_ps")
        for _ in range(N_WARM):
            nc.tensor.matmul(warm_ps[:], warm_sb[:], warm_sb[:, :64],
                             start=True, stop=True, skip_group_check=True)

        # Input loads. Weights go on the sync HWDGE ring, token chunks on the
        # scalar ring: the two rings issue their (~0.6us/instr) descriptor
        # writes in parallel and deliver independently, so x0 is not queued
        # behind 2 MB of w1.
        w1h_sb = wpool.tile([128, 2 * ND * 128], dt.bfloat16, tag="w1h",
                            name="w1h_sb")
        nc.sync.dma_start(w1h_sb[:], w1h[:])
        w1b_sb = wpool.tile([128, 2 * ND * 128], dt.bfloat16, tag="w1b",
                            name="w1b_sb")
        nc.sync.dma_start(w1b_sb[:], w1b[:])
        w1r_sb = {}
        for hb in range(3):
            t = wpool.tile([128, 4 * ND * 128], dt.bfloat16,
                           tag=f"w1r_{hb}", name=f"w1r_{hb}")
            o = hb * 4 * ND * 128
            nc.sync.dma_start(t[:], w1r[:, o:o + 4 * ND * 128])
            w1r_sb[hb] = t
        w2_sb = []
        for wb in range(2):
            t = wpool.tile([128, 8 * 512], dt.bfloat16,
                           tag=f"w2_{wb}", name=f"w2_{wb}")
            o = wb * 8 * 512
            nc.sync.dma_start(t[:], w2[:, o:o + 8 * 512])
            w2_sb.append(t)

        xt_sb = {}
        for (c0, S) in chunks:
            t = xpool.tile([128, ND * S], dt.bfloat16,
                           tag=f"xt_{c0}", name=f"xt_{c0}")
            o = xy_off[c0]
            nc.scalar.dma_start(t[:], xt[:, o:o + ND * S])
            xt_sb[c0] = t

        if not fold_gate:
            b1_sb = wpool.tile([128, NH], dt.float32, tag="b1", name="b1_sb")
            nc.scalar.dma_start(b1_sb[:], b1r[:])
            b2_sb = wpool.tile([128, ND], dt.float32, tag="b2", name="b2_sb")
            nc.scalar.dma_start(b2_sb[:], b2r[:])
            gr_sb = wpool.tile([128, C], dt.float32, tag="gr", name="gr_sb")
            nc.scalar.dma_start(gr_sb[:], gr[:])

        def w1_lhsT(ht, di):
            if ht < 2:
                return w1h_sb[:, ht * 512 + di * 128:ht * 512 + (di + 1) * 128]
            if ht < 4:
                o = (ht - 2) * 512 + di * 128
                return w1b_sb[:, o:o + 128]
            o = ((ht - 4) % 4) * 512 + di * 128
            return w1r_sb[(ht - 4) // 4][:, o:o + 128]

        def gemm1(c0, S):
            # mid^T[h, c] = relu(sum_d w1[d,h] * x^T[d,c] (+ b1[h]))
            mids = []
            for ht in range(NH):
                ps = p1.tile([128, S], dt.float32, tag="ps1", name=f"ps1_{c0}_{ht}")
                for di in range(ND):
                    nc.tensor.matmul(
                        ps[:],
                        w1_lhsT(ht, di),
                        xt_sb[c0][:, di * S:(di + 1) * S],
                        start=(di == 0),
                        stop=(di == ND - 1),
                    )
                m = midp.tile([128, S], dt.bfloat16, tag="mid", name=f"mid_{c0}_{ht}")
                if fold_gate:
                    nc.scalar.activation(m[:], ps[:], AF.Relu)
                else:
                    nc.scalar.activation(m[:], ps[:], AF.Relu,
                                         bias=b1_sb[:, ht:ht + 1])
                mids.append(m)
            return mids

        def gemm2(c0, S, mids):
            # y^T[d, c] = (sum_h w2[h,d] * mid^T[h,c] (+ b2[d])) (* g[c])
            o = xy_off[c0]
            for di in range(ND):
                ps2 = p2.tile([128, S], dt.float32, tag="ps2", name=f"ps2_{c0}_{di}")
                for ht in range(NH):
                    wo = (ht % 8) * 512 + di * 128
                    nc.tensor.matmul(
                        ps2[:],
                        w2_sb[ht // 8][:, wo:wo + 128],
                        mids[ht][:],
                        start=(ht == 0),
                        stop=(ht == NH - 1),
                    )
                yt_t = ypool.tile([128, S], dt.bfloat16, tag="y", name=f"y_{c0}_{di}")
                if fold_gate:
                    nc.vector.tensor_copy(yt_t[:], ps2[:])
                else:
                    nc.vector.tensor_mul(yt_t[:], ps2[:], gr_sb[:, c0:c0 + S])
                nc.sync.dma_start(yt[:, o + di * S:o + (di + 1) * S], yt_t[:])

        # software-pipeline by one chunk: GEMM1 of chunk i+1 is emitted before
        # GEMM2 of chunk i, giving the PE dense work while w2 streams in
        prev = None
        for (c0, S) in chunks:
            mids = gemm1(c0, S)
            if prev is not None:
                gemm2(*prev)
            prev = (c0, S, mids)
        gemm2(*prev)

    nc.finalize()
    return nc


def _route(h, w_gate):
    """Top-2 gating, matching jax.lax.top_k (ties -> lower index) + softmax."""
    logits = h @ w_gate                                      # [N, E] f32
    order = np.argsort(-logits, axis=1, kind="stable")
    top_idx = order[:, :TOP_K]                               # [N, 2]
    top_lg = np.take_along_axis(logits, top_idx, axis=1)
    mx = top_lg.max(axis=1, keepdims=True)
    ex = np.exp(top_lg - mx)
    gates2 = (ex / ex.sum(axis=1, keepdims=True)).astype(np.float32)
    return top_idx, gates2


def _run(inputs, trace=False):
    from concourse.bass_utils import run_bass_kernel_spmd

    bf16 = ml_dtypes.bfloat16
    h = np.asarray(inputs["h"], dtype=np.float32)
    w_gate = np.asarray(inputs["w_gate"], dtype=np.float32)
    w1 = np.asarray(inputs["w1"], dtype=np.float32)
    b1 = np.asarray(inputs["b1"], dtype=np.float32)
    w2 = np.asarray(inputs["w2"], dtype=np.float32)
    b2 = np.asarray(inputs["b2"], dtype=np.float32)
    N = h.shape[0]

    fold_gate = not (b1.any() or b2.any())
    top_idx, gates2 = _route(h, w_gate)

    # dispatch: expert e -> cores 2e (first half) and 2e+1 (second half),
    # fixed capacity C per core. Over-capacity pairs (lowest gates first)
    # fall back to an exact host-side residual; under-capacity is zero-pad.
    core_toks, core_gates, core_expert = [], [], []
    resid = []                                              # (expert, toks, gates)
    for e in range(E):
        sel = top_idx == e                                   # [N, 2] bool
        toks = np.nonzero(sel.any(axis=1))[0]
        g = gates2[toks, sel[toks].argmax(axis=1)]
        if len(toks) > 2 * C:
            keep = np.sort(np.argsort(-g, kind="stable")[:2 * C])
            spill = np.sort(np.argsort(-g, kind="stable")[2 * C:])
            resid.append((e, toks[spill], g[spill]))
            toks, g = toks[keep], g[keep]
        half = (len(toks) + 1) // 2
        for lo, hi in ((0, half), (half, len(toks))):
            core_toks.append(toks[lo:hi])
            core_gates.append(g[lo:hi])
            core_expert.append(e)

    key = fold_gate
    if key not in _NC_CACHE:
        _NC_CACHE[key] = _build_moe_nc(fold_gate)
    nc = _NC_CACHE[key]

    # partition-major packers matching the kernel's flat DMA layouts
    def pack_w1(e, ht0, ht1):
        # [128, (ht, di, 128)] h-tile-major
        return np.ascontiguousarray(
            w1[e].astype(bf16).reshape(ND, 128, NH, 128)
            .transpose(1, 2, 0, 3)[:, ht0:ht1].reshape(128, (ht1 - ht0) * ND * 128))

    w1h_p, w1b_p, w1r_p, w2_p = {}, {}, {}, {}
    for e in set(core_expert):
        w1h_p[e] = pack_w1(e, 0, 2)
        w1b_p[e] = pack_w1(e, 2, 4)
        w1r_p[e] = pack_w1(e, 4, 16)
        w2_p[e] = np.ascontiguousarray(
            w2[e].astype(bf16).reshape(2, 8, 128, 512)
            .transpose(2, 0, 1, 3).reshape(128, 2 * 8 * 512))

    in_maps = []
    for c in range(N_CORES):
        e = core_expert[c]
        toks = core_toks[c]
        n = len(toks)
        xtT = np.zeros((D, C), dtype=bf16)
        if fold_gate:
            xtT[:, :n] = (h[toks] * core_gates[c][:, None]).T.astype(bf16)
        else:
            xtT[:, :n] = h[toks].T.astype(bf16)
        r = xtT.reshape(ND, 128, C)
        xt_arr = np.empty((128, ND * C), dtype=bf16)
        o = 0
        for (c0, S) in CHUNKS:
            xt_arr[:, o:o + ND * S] = (
                r[:, :, c0:c0 + S].transpose(1, 0, 2).reshape(128, ND * S))
            o += ND * S
        im = {
            "w1h": w1h_p[e],
            "w1b": w1b_p[e],
            "w1r": w1r_p[e],
            "w2": w2_p[e],
            "xt": xt_arr,
        }
        if not fold_gate:
            grow = np.zeros(C, dtype=np.float32)
            grow[:n] = core_gates[c]
            im["b1r"] = np.ascontiguousarray(b1[e].reshape(NH, 128).T)
            im["b2r"] = np.ascontiguousarray(b2[e].reshape(ND, 128).T)
            im["gr"] = np.ascontiguousarray(np.broadcast_to(grow, (128, C)))
        in_maps.append(im)

    res = run_bass_kernel_spmd(nc, in_maps, core_ids=list(range(N_CORES)),
                               trace=trace)

    out = np.zeros((N, D), dtype=np.float32)
    for c in range(N_CORES):
        toks = core_toks[c]
        if not len(toks):
            continue
        # unpack chunk-major [128, ND*C] back to y^T [D, C]
        raw = np.asarray(res.results[c]["yt"], dtype=np.float32)
        ytT = np.empty((D, C), dtype=np.float32)
        o = 0
        for (c0, S) in CHUNKS:
            ytT[:, c0:c0 + S] = (
                raw[:, o:o + ND * S].reshape(128, ND, S)
                .transpose(1, 0, 2).reshape(D, S))
            o += ND * S
        out[toks] += ytT[:, :len(toks)].T
    # exact host residual for over-capacity pairs (rare, ~tens of tokens)
    for (e, toks, g) in resid:
        mid = np.maximum(h[toks] @ w1[e] + b1[e], 0.0)
        out[toks] += g[:, None] * (mid @ w2[e] + b2[e])
    return out, res


def kernel(**inputs) -> np.ndarray:
    out, _ = _run(inputs, trace=False)
    return out


# revision 4
# speedup vs baseline: 1.0175x; 1.0175x over previous
"""MoE FeedForward (top-2 of 4 experts) — expert-parallel Trainium2 kernel.

Strategy (matches the sharding hint): the tiny gating matmul + top-k routing
run on host as part of input sharding; tokens are dispatched by gate index to
expert-owning cores (expert e -> cores 2e, 2e+1). Each core computes
    y^T = (relu(W1^T @ x^T + b1) -> W2^T @ mid + b2) * gate
entirely in transposed [feature, token] layout (no on-device transposes),
with bf16 matmuls accumulating in fp32 PSUM. The host combine scatter-adds
the two gate-weighted expert contributions per token.

Capacity is FIXED at C=2048 columns per core (the exact balanced share of
N_TOKENS*TOP_K/8): experts with more than 2C routed pairs spill their
lowest-gate pairs to a tiny host-side numpy residual (exact, ~tens of pairs),
lighter experts are zero-padded. A single compiled program therefore serves
every input and every core runs identical, perfectly balanced work.

Fast path (used whenever b1 == b2 == 0, which holds for this problem): the
positive gate is folded into x on host (relu(g*z) == g*relu(z) for g > 0),
removing the on-device gate multiply and its [128, C] gate-broadcast load.

Per-core schedule: chunks of [256, 512, 512, 512, 256] columns. The small
first chunk needs only 0.5 MB (x0 + first two w1 h-tiles) before matmuls can
start, and its weight consumption rate stays below DMA delivery during the
bandwidth-saturated head; the small last chunk shrinks the drain tail.
Weights stream on the sync DMA ring, activations on the scalar ring and
outputs (bf16) on the vector ring so issue latencies overlap. w1 is packed
h-tile-major so arrival order equals consumption order.

Model dims (hardcoded per problem spec): N=8192 tokens, D=512, H=2048,
E=4 experts, top-k=2, 8 NeuronCores.
"""

import numpy as np
import ml_dtypes
from contextlib import ExitStack

D = 512
H = 2048
E = 4
TOP_K = 2
N_CORES = 8
ND = D // 128    # 4 d-tiles
NH = H // 128    # 16 h-tiles
C = 2048         # fixed per-core token capacity
CHUNKS = [(0, 256), (256, 512), (768, 512), (1280, 512), (1792, 256)]
N_WARM = 40      # PE warm-up matmuls (cover preamble->first-data window)

_NC_CACHE = {}


def _build_moe_nc(fold_gate: bool):
    """Per-core SPMD program: [D,C] bf16 tokens -> [D,C] bf16 expert output."""
    import concourse.mybir as mybir
    from concourse import bacc, tile

    dt = mybir.dt
    AF = mybir.ActivationFunctionType

    chunks = CHUNKS
    nc = bacc.Bacc(None, target_bir_lowering=False)

    # host pre-arranges every input partition-major so each DMA below is a
    # flat, fully contiguous [128, K] copy (max SDMA bandwidth, min
    # descriptor count):
    #   w1h/w1b/w1r: h-tile-major blocks, per ht: [128, ND*128] (di-major)
    #     -> arrival order == gemm1 consumption order (ht ascending)
    #   xt: chunk-major blocks, inside a block di-major: [128, ND*S]
    #   w2: wb-major blocks of [128, 8*512]
    w1h = nc.dram_tensor("w1h", [128, 2 * ND * 128], dt.bfloat16,
                         kind="ExternalInput")           # ht 0-1
    w1b = nc.dram_tensor("w1b", [128, 2 * ND * 128], dt.bfloat16,
                         kind="ExternalInput")           # ht 2-3
    w1r = nc.dram_tensor("w1r", [128, 12 * ND * 128], dt.bfloat16,
                         kind="ExternalInput")           # ht 4-15
    w2 = nc.dram_tensor("w2", [128, 2 * 8 * 512], dt.bfloat16,
                        kind="ExternalInput")
    xt = nc.dram_tensor("xt", [128, ND * C], dt.bfloat16,
                        kind="ExternalInput")
    if not fold_gate:
        b1r = nc.dram_tensor("b1r", [128, NH], dt.float32, kind="ExternalInput")
        b2r = nc.dram_tensor("b2r", [128, ND], dt.float32, kind="ExternalInput")
        gr = nc.dram_tensor("gr", [128, C], dt.float32, kind="ExternalInput")
    # output, chunk-major like xt: per chunk a [128, ND*S] contiguous block
    # (one DMA per (chunk, di)); host unpacks back to [N, D] token rows
    yt = nc.dram_tensor("yt", [128, ND * C], dt.bfloat16, kind="ExternalOutput")

    # flat offset of each chunk's block inside xt / yt
    xy_off = {}
    acc = 0
    for (c0, S) in chunks:
        xy_off[c0] = acc
        acc += ND * S

    with tile.TileContext(nc) as tc, ExitStack() as ctx:
        wpool = ctx.enter_context(tc.tile_pool(name="weights", bufs=1))
        xpool = ctx.enter_context(tc.tile_pool(name="x", bufs=1))
        midp = ctx.enter_context(tc.tile_pool(name="mid", bufs=34))
        p1 = ctx.enter_context(tc.tile_pool(name="p1", bufs=4, space="PSUM"))
        p2 = ctx.enter_context(tc.tile_pool(name="p2", bufs=3, space="PSUM"))
        pw = ctx.enter_context(tc.tile_pool(name="pw", bufs=1, space="PSUM"))
        ypool = ctx.enter_context(tc.tile_pool(name="y", bufs=6))

        # PE warm-up: dummy matmuls spanning the engine preamble + first
        # input DMAs so the HAM clock gate is at full rate when real matmuls
        # begin. Output bank is never read.
        warm_sb = wpool.tile([128, 128], dt.bfloat16, tag="warm", name="warm_sb")
        nc.vector.memset(warm_sb[:], 0.0)
        warm_ps = pw.tile([128, 64], dt.float32, tag="warm_ps", name="warm_ps")
        for _ in range(N_WARM):
            nc.tensor.matmul(warm_ps[:], warm_sb[:], warm_sb[:, :64],
                             start=True, stop=True, skip_group_check=True)

        # Input loads: ONE ring (sync HWDGE), in exact consumption order.
        # The 16 SDMA engines round-robin across active rings, so a second
        # ring would steal bandwidth from the critical head stream; a single
        # FIFO ring delivers x0 -> w1 (ht ascending) -> x1 -> w2 -> x2..x4
        # just in time while gemm1(chunk0) self-paces against delivery.
        xt_sb = {}

        def load_xt(c0, S):
            t = xpool.tile([128, ND * S], dt.bfloat16,
                           tag=f"xt_{c0}", name=f"xt_{c0}")
            o = xy_off[c0]
            nc.sync.dma_start(t[:], xt[:, o:o + ND * S])
            xt_sb[c0] = t

        load_xt(*chunks[0])
        w1h_sb = wpool.tile([128, 2 * ND * 128], dt.bfloat16, tag="w1h",
                            name="w1h_sb")
        nc.sync.dma_start(w1h_sb[:], w1h[:])
        w1b_sb = wpool.tile([128, 2 * ND * 128], dt.bfloat16, tag="w1b",
                            name="w1b_sb")
        nc.sync.dma_start(w1b_sb[:], w1b[:])
        w1r_sb = {}
        for hb in range(3):
            t = wpool.tile([128, 4 * ND * 128], dt.bfloat16,
                           tag=f"w1r_{hb}", name=f"w1r_{hb}")
            o = hb * 4 * ND * 128
            nc.sync.dma_start(t[:], w1r[:, o:o + 4 * ND * 128])
            w1r_sb[hb] = t
        load_xt(*chunks[1])
        w2_sb = []
        for wb in range(2):
            t = wpool.tile([128, 8 * 512], dt.bfloat16,
                           tag=f"w2_{wb}", name=f"w2_{wb}")
            o = wb * 8 * 512
            nc.sync.dma_start(t[:], w2[:, o:o + 8 * 512])
            w2_sb.append(t)
            if wb == 0:
                load_xt(*chunks[2])
        load_xt(*chunks[3])
        load_xt(*chunks[4])

        if not fold_gate:
            b1_sb = wpool.tile([128, NH], dt.float32, tag="b1", name="b1_sb")
            nc.scalar.dma_start(b1_sb[:], b1r[:])
            b2_sb = wpool.tile([128, ND], dt.float32, tag="b2", name="b2_sb")
            nc.scalar.dma_start(b2_sb[:], b2r[:])
            gr_sb = wpool.tile([128, C], dt.float32, tag="gr", name="gr_sb")
            nc.scalar.dma_start(gr_sb[:], gr[:])

        def w1_lhsT(ht, di):
            if ht < 2:
                return w1h_sb[:, ht * 512 + di * 128:ht * 512 + (di + 1) * 128]
            if ht < 4:
                o = (ht - 2) * 512 + di * 128
                return w1b_sb[:, o:o + 128]
            o = ((ht - 4) % 4) * 512 + di * 128
            return w1r_sb[(ht - 4) // 4][:, o:o + 128]

        def gemm1(c0, S):
            # mid^T[h, c] = relu(sum_d w1[d,h] * x^T[d,c] (+ b1[h]))
            mids = []
            for ht in range(NH):
                ps = p1.tile([128, S], dt.float32, tag="ps1", name=f"ps1_{c0}_{ht}")
                for di in range(ND):
                    nc.tensor.matmul(
                        ps[:],
                        w1_lhsT(ht, di),
                        xt_sb[c0][:, di * S:(di + 1) * S],
                        start=(di == 0),
                        stop=(di == ND - 1),
                    )
                m = midp.tile([128, S], dt.bfloat16, tag="mid", name=f"mid_{c0}_{ht}")
                if fold_gate:
                    nc.scalar.activation(m[:], ps[:], AF.Relu)
                else:
                    nc.scalar.activation(m[:], ps[:], AF.Relu,
                                         bias=b1_sb[:, ht:ht + 1])
                mids.append(m)
            return mids

        def gemm2(c0, S, mids):
            # y^T[d, c] = (sum_h w2[h,d] * mid^T[h,c] (+ b2[d])) (* g[c])
            o = xy_off[c0]
            for di in range(ND):
                ps2 = p2.tile([128, S], dt.float32, tag="ps2", name=f"ps2_{c0}_{di}")
                for ht in range(NH):
                    wo = (ht % 8) * 512 + di * 128
                    nc.tensor.matmul(
                        ps2[:],
                        w2_sb[ht // 8][:, wo:wo + 128],
                        mids[ht][:],
                        start=(ht == 0),
                        stop=(ht == NH - 1),
                    )
                yt_t = ypool.tile([128, S], dt.bfloat16, tag="y", name=f"y_{c0}_{di}")
                if fold_gate:
                    nc.vector.tensor_copy(yt_t[:], ps2[:])
                else:
                    nc.vector.tensor_mul(yt_t[:], ps2[:], gr_sb[:, c0:c0 + S])
                nc.sync.dma_start(yt[:, o + di * S:o + (di + 1) * S], yt_t[:])

        # software-pipeline by one chunk: GEMM1 of chunk i+1 is emitted before
        # GEMM2 of chunk i, giving the PE dense work while w2 streams in
        prev = None
        for (c0, S) in chunks:
            mids = gemm1(c0, S)
            if prev is not None:
                gemm2(*prev)
            prev = (c0, S, mids)
        gemm2(*prev)

    nc.finalize()
    return nc


def _route(h, w_gate):
    """Top-2 gating, matching jax.lax.top_k (ties -> lower index) + softmax."""
    logits = h @ w_gate                                      # [N, E] f32
    order = np.argsort(-logits, axis=1, kind="stable")
    top_idx = order[:, :TOP_K]                               # [N, 2]
    top_lg = np.take_along_axis(logits, top_idx, axis=1)
    mx = top_lg.max(axis=1, keepdims=True)
    ex = np.exp(top_lg - mx)
    gates2 = (ex / ex.sum(axis=1, keepdims=True)).astype(np.float32)
    return top_idx, gates2


def _run(inputs, trace=False):
    from concourse.bass_utils import run_bass_kernel_spmd

    bf16 = ml_dtypes.bfloat16
    h = np.asarray(inputs["h"], dtype=np.float32)
    w_gate = np.asarray(inputs["w_gate"], dtype=np.float32)
    w1 = np.asarray(inputs["w1"], dtype=np.float32)
    b1 = np.asarray(inputs["b1"], dtype=np.float32)
    w2 = np.asarray(inputs["w2"], dtype=np.float32)
    b2 = np.asarray(inputs["b2"], dtype=np.float32)
    N = h.shape[0]

    fold_gate = not (b1.any() or b2.any())
    top_idx, gates2 = _route(h, w_gate)

    # dispatch: expert e -> cores 2e (first half) and 2e+1 (second half),
    # fixed capacity C per core. Over-capacity pairs (lowest gates first)
    # fall back to an exact host-side residual; under-capacity is zero-pad.
    core_toks, core_gates, core_expert = [], [], []
    resid = []                                              # (expert, toks, gates)
    for e in range(E):
        sel = top_idx == e                                   # [N, 2] bool
        toks = np.nonzero(sel.any(axis=1))[0]
        g = gates2[toks, sel[toks].argmax(axis=1)]
        if len(toks) > 2 * C:
            keep = np.sort(np.argsort(-g, kind="stable")[:2 * C])
            spill = np.sort(np.argsort(-g, kind="stable")[2 * C:])
            resid.append((e, toks[spill], g[spill]))
            toks, g = toks[keep], g[keep]
        half = (len(toks) + 1) // 2
        for lo, hi in ((0, half), (half, len(toks))):
            core_toks.append(toks[lo:hi])
            core_gates.append(g[lo:hi])
            core_expert.append(e)

    key = fold_gate
    if key not in _NC_CACHE:
        _NC_CACHE[key] = _build_moe_nc(fold_gate)
    nc = _NC_CACHE[key]

    # partition-major packers matching the kernel's flat DMA layouts
    def pack_w1(e, ht0, ht1):
        # [128, (ht, di, 128)] h-tile-major
        return np.ascontiguousarray(
            w1[e].astype(bf16).reshape(ND, 128, NH, 128)
            .transpose(1, 2, 0, 3)[:, ht0:ht1].reshape(128, (ht1 - ht0) * ND * 128))

    w1h_p, w1b_p, w1r_p, w2_p = {}, {}, {}, {}
    for e in set(core_expert):
        w1h_p[e] = pack_w1(e, 0, 2)
        w1b_p[e] = pack_w1(e, 2, 4)
        w1r_p[e] = pack_w1(e, 4, 16)
        w2_p[e] = np.ascontiguousarray(
            w2[e].astype(bf16).reshape(2, 8, 128, 512)
            .transpose(2, 0, 1, 3).reshape(128, 2 * 8 * 512))

    in_maps = []
    for c in range(N_CORES):
        e = core_expert[c]
        toks = core_toks[c]
        n = len(toks)
        xtT = np.zeros((D, C), dtype=bf16)
        if fold_gate:
            xtT[:, :n] = (h[toks] * core_gates[c][:, None]).T.astype(bf16)
        else:
            xtT[:, :n] = h[toks].T.astype(bf16)
        r = xtT.reshape(ND, 128, C)
        xt_arr = np.empty((128, ND * C), dtype=bf16)
        o = 0
        for (c0, S) in CHUNKS:
            xt_arr[:, o:o + ND * S] = (
                r[:, :, c0:c0 + S].transpose(1, 0, 2).reshape(128, ND * S))
            o += ND * S
        im = {
            "w1h": w1h_p[e],
            "w1b": w1b_p[e],
            "w1r": w1r_p[e],
            "w2": w2_p[e],
            "xt": xt_arr,
        }
        if not fold_gate:
            grow = np.zeros(C, dtype=np.float32)
            grow[:n] = core_gates[c]
            im["b1r"] = np.ascontiguousarray(b1[e].reshape(NH, 128).T)
            im["b2r"] = np.ascontiguousarray(b2[e].reshape(ND, 128).T)
            im["gr"] = np.ascontiguousarray(np.broadcast_to(grow, (128, C)))
        in_maps.append(im)

    res = run_bass_kernel_spmd(nc, in_maps, core_ids=list(range(N_CORES)),
                               trace=trace)

    out = np.zeros((N, D), dtype=np.float32)
    for c in range(N_CORES):
        toks = core_toks[c]
        if not len(toks):
            continue
        # unpack chunk-major [128, ND*C] back to y^T [D, C]
        raw = np.asarray(res.results[c]["yt"], dtype=np.float32)
        ytT = np.empty((D, C), dtype=np.float32)
        o = 0
        for (c0, S) in CHUNKS:
            ytT[:, c0:c0 + S] = (
                raw[:, o:o + ND * S].reshape(128, ND, S)
                .transpose(1, 0, 2).reshape(D, S))
            o += ND * S
        out[toks] += ytT[:, :len(toks)].T
    # exact host residual for over-capacity pairs (rare, ~tens of tokens)
    for (e, toks, g) in resid:
        mid = np.maximum(h[toks] @ w1[e] + b1[e], 0.0)
        out[toks] += g[:, None] * (mid @ w2[e] + b2[e])
    return out, res


def kernel(**inputs) -> np.ndarray:
    out, _ = _run(inputs, trace=False)
    return out
